# revision 72
# baseline (speedup 1.0000x reference)
"""Trainium2 Bass kernel for NnqlmCnnBasedLstm.

Math (per batch item, per input sequence q/a):
  xe = embed[idx]                      (L, D)       D = 128
  dens_t = outer(xe_t, xe_t)/(|xe_t|^2 + 1e-4)     (D, D), symmetric
  2-layer ConvLSTM over L=40 steps; each gate g:
    pre_g = conv2d([xt; h], W_g, stride=(2,1), pad=(1,1)) + b_g  on (2D, D) -> (D, D)
  c = sig(f)*c + sig(i)*tanh(cc); h = sig(o)*tanh(c)
  out = max_t h2_t  -> flatten -> concat(q,a) -> linear(2) -> log_softmax

Device strategy (8 cores, data parallel over B=32 -> 4 items/core, each with a
q-chain and an a-chain = 8 chains/core):
  * State kept TRANSPOSED: tiles are (w partitions, j free); densities are
    symmetric so layer-1 inputs need no transpose.
  * conv: out_T[w, j] = sum_{dh,dw} W[dh,dw] * inp_T[w-1+dw, 2j-1+dh].
    For each dh this is a 3-diagonal Toeplitz band matrix (over w) applied on
    the TensorEngine; the four dh taps are packed as two fp8 DoubleRow
    matmuls (dh pairs (0,1) and (2,3) read 16-bit-aligned byte pairs of the
    input, the paired band matrices are the stationary).
  * Software pipeline: layer 2 lags layer 1 by one time step, so every
    cross-engine dependency has about a full step of slack and the PE
    streams matmuls gap-free (HAM stays at full clock).
  * sigmoid/tanh on ScalarE in full-width (1024 col) instructions; cell
    updates on VectorE in bf16; densities via a DMA row-broadcast of the
    embedding vectors plus per-chain VectorE tensor_scalar outer products;
    h1 fan-out copy on GpSimd.
  * Embedding gather, final linear + log_softmax on host (tiny).
"""

import os
import sys

import numpy as np
import ml_dtypes

for _p in ("/opt/trn_rl_repo", "/root/.axon_site/_ro/trn_rl_repo"):
    if os.path.isdir(_p) and _p not in sys.path:
        sys.path.insert(0, _p)

BF16 = np.dtype(ml_dtypes.bfloat16)
F8 = np.dtype(ml_dtypes.float8_e4m3)
DEFER_T2 = os.environ.get("KERNEL_DEFER_T2", "0") == "1"
SIM_MM = os.environ.get("KERNEL_SIM_MM", "0") == "1"

B, L, D, V, NL = 32, 40, 128, 32000, 2
NCORES = 8
CH = 8            # chains per core: 4 batch items x {q, a}
GW = 4            # chains per matmul group (psum free-width limit)
SEG = 258         # [z x(128) h(128) z]; dh-pair reads start at even elems
XOFF, HOFF = 1, 129
NF = CH * SEG
GF = GW * D       # free width of one matmul group (512)
FW = CH * D       # full free width (1024)

_CACHE = {}


def _build_nc(L=L):
    import concourse.bass as bass
    import concourse.bacc as bacc
    import concourse.mybir as mybir
    from concourse import tile

    f32 = mybir.dt.float32
    bf16 = mybir.dt.bfloat16
    f8 = mybir.dt.float8e4
    AF = mybir.ActivationFunctionType
    ALU = mybir.AluOpType
    DR = mybir.MatmulPerfMode.DoubleRow

    nc = bacc.Bacc(None, target_bir_lowering=False)

    # host-precomputed densities outer(y_s, y_s): (L, D, CH*D) fp8
    xdens_d = nc.dram_tensor("xdens", (L, D, FW), f8, kind="ExternalInput")
    # band stationaries as dh-pairs for DoubleRow: slot k=(l*4+g)*2+pr holds
    # [B_{2pr}^T ; B_{2pr+1}^T] as (D, two, D)
    st_d = nc.dram_tensor("st", (NL * 4 * 2, D, 2 * D), f8, kind="ExternalInput")
    bias_d = nc.dram_tensor("bias", (D, NL * 4), f32, kind="ExternalInput")
    zpad_d = nc.dram_tensor("zpad", (D, CH * 2), f8, kind="ExternalInput")
    out_d = nc.dram_tensor("mp_out", (D, FW), bf16, kind="ExternalOutput")

    GORDER = [3, 1, 0, 2]          # conv_w gate order: cc, i, f, o
    GTAG = {2: "po", 0: "pf", 1: "pi", 3: "pc"}

    with tile.TileContext(nc) as tc:
        with (
            tc.tile_pool(name="const", bufs=1) as constp,
            tc.tile_pool(name="state", bufs=1) as statep,
            tc.tile_pool(name="inp0", bufs=2) as inp0p,
            tc.tile_pool(name="inp1", bufs=2) as inp1p,
            tc.tile_pool(name="gate", bufs=2) as gatep,
            tc.tile_pool(name="psum", bufs=1, space="PSUM") as psump,
        ):
            # ---- constants ----
            stT = constp.tile([D, NL * 4 * 2 * 2 * D], f8, tag="stT")
            for i in range(NL * 4 * 2):
                nc.sync.dma_start(stT[:, i * 2 * D:(i + 1) * 2 * D], st_d[i])
            bias = constp.tile([D, NL * 4], f32, tag="bias")
            nc.sync.dma_start(bias[:], bias_d[:])

            # ---- persistent state ----
            c_l = [statep.tile([D, FW], bf16, tag=f"c{l}", name=f"c{l}")
                   for l in range(NL)]
            mp = statep.tile([D, FW], bf16, tag="mp")
            for l in range(NL):
                nc.vector.memset(c_l[l][:], 0.0)
            nc.vector.memset(mp[:], -1e30)

            def seg3(t):
                return t[:].rearrange("p (s c) -> p s c", s=CH)

            def xpart(t):
                return seg3(t)[:, :, XOFF:XOFF + D]

            def hpart(t):
                return seg3(t)[:, :, HOFF:HOFF + D]

            def new_inp(pool, tag, zero_pads):
                t = pool.tile([D, NF], f8, tag=tag, name=tag)
                if zero_pads:
                    v = seg3(t)
                    nc.sync.dma_start(v[:, :, 0:1], zpad_d[:, 0:CH])
                    nc.sync.dma_start(v[:, :, SEG - 1:SEG], zpad_d[:, 0:CH])
                return t

            def densities(t, dst):
                """dst x-part <- host-precomputed outer(y_s, y_s) via DMA."""
                nc.sync.dma_start(xpart(dst), xdens_d[t])

            def band_mms(l, inp):
                """Gate pre-activation DoubleRow matmuls, both chain groups."""
                # (p, two, s, c): elem = s*SEG + 2c + two; output j for pair
                # pr reads elems (2j+2pr, 2j+2pr+1)
                i4p = seg3(inp).rearrange("p s (c two) -> p two s c", two=2)
                ps = {}
                for g in GORDER:
                    p = psump.tile([D, FW], f32, tag=GTAG[g], name=GTAG[g])
                    ps[g] = p
                    for grp in range(2):
                        gs = slice(grp * GW, (grp + 1) * GW)
                        for pr in range(2):
                            k = (l * 4 + g) * 2 + pr
                            lhsT = stT[:, k * 2 * D:(k + 1) * 2 * D].rearrange(
                                "p (two m) -> p two m", two=2)
                            if not SIM_MM:
                                nc.tensor.matmul(
                                    p[:, grp * GF:(grp + 1) * GF],
                                    lhsT, i4p[:, :, gs, pr:pr + D],
                                    start=(pr == 0), stop=(pr == 1),
                                    perf_mode=DR,
                                )
                        if SIM_MM:
                            # per-chain 3D rhs (CoreSim-compatible); chain
                            # outer / pr inner so each 2KB psum zero region
                            # has one open accumulation group at a time
                            for s in range(GW):
                                ch = grp * GW + s
                                r3 = inp[:, ch * SEG:(ch + 1) * SEG] \
                                    .rearrange("p (c two) -> p two c", two=2)
                                for pr in range(2):
                                    k = (l * 4 + g) * 2 + pr
                                    lhsT = stT[:, k * 2 * D:(k + 1) * 2 * D] \
                                        .rearrange("p (two m) -> p two m",
                                                   two=2)
                                    nc.tensor.matmul(
                                        p[:, ch * D:(ch + 1) * D],
                                        lhsT, r3[:, :, pr:pr + D],
                                        start=(pr == 0), stop=(pr == 1),
                                        perf_mode=DR,
                                    )
                return ps

            def gate_acts(l, ps):
                gt = {}
                for g, af in ((3, AF.Tanh), (1, AF.Sigmoid),
                              (0, AF.Sigmoid), (2, AF.Sigmoid)):
                    dst = gatep.tile([D, FW], bf16, tag=f"g{g}", name=f"g{g}")
                    nc.scalar.activation(dst[:], ps[g][:], af,
                                         bias=bias[:, l * 4 + g:l * 4 + g + 1])
                    gt[g] = dst
                return gt

            def cell_pre(l, gt):
                t1 = gatep.tile([D, FW], bf16, tag="t1")
                t2 = gatep.tile([D, FW], bf16, tag="t2")
                nc.vector.tensor_mul(t2[:], gt[1][:], gt[3][:])
                nc.vector.tensor_mul(t1[:], gt[0][:], c_l[l][:])
                nc.vector.tensor_add(c_l[l][:], t1[:], t2[:])

            def cell_tanh(l):
                th = gatep.tile([D, FW], bf16, tag="th")
                nc.scalar.activation(th[:], c_l[l][:], AF.Tanh)
                return th

            # ---- prologue ----
            t0 = new_inp(inp0p, "inp0", True)         # L1 input for step 0
            nc.vector.memset(hpart(t0), 0.0)          # h1_{-1} = 0
            densities(0, t0)
            tiles0 = {0: t0}
            tiles1 = {}

            # ---- wall-clock loop: L1 computes step t, L2 computes t-1;
            # L2's tanh/h-store is deferred to the NEXT wall step ----
            pend2 = None             # (og tile, h2 dest ap) for L2 step t-2

            def flush_pend2():
                """tanh + h2 store + maxpool for the pending L2 step."""
                og2, h2dst = pend2
                th2 = cell_tanh(1)
                h2b = gatep.tile([D, FW], bf16, tag="h2b")
                nc.vector.tensor_mul(h2b[:], og2, th2[:])
                nc.vector.tensor_tensor(mp[:], mp[:], h2b[:], op=ALU.max)
                if h2dst is not None:
                    # fp8 cast into the L2 input tile (read next wall-step):
                    # gpsimd cast-DMA, off the compute engines
                    nc.gpsimd.dma_start(h2dst, h2b[:])

            for t in range(L + 1):
                do1 = t < L          # L1 window for step t
                do2 = t >= 1         # L2 window for step t - 1
                s2 = t - 1

                if do1:
                    tiles1[t] = new_inp(inp1p, "inp1", True)
                    if t == 0:
                        nc.vector.memset(hpart(tiles1[0]), 0.0)  # h2_{-1}=0

                if pend2 is not None and DEFER_T2:
                    flush_pend2()
                    pend2 = None

                if do1 and t + 1 < L:
                    tiles0[t + 1] = new_inp(inp0p, "inp0", True)
                    densities(t + 1, tiles0[t + 1])

                # --- PE: L1(t) then L2(t-1) ---
                ps1 = band_mms(0, tiles0[t]) if do1 else None
                ps2 = band_mms(1, tiles1[s2]) if do2 else None

                # --- L1(t) acts + cell + h1 ---
                if do1:
                    gt1 = gate_acts(0, ps1)
                    cell_pre(0, gt1)
                    th1 = cell_tanh(0)
                    h1b = gatep.tile([D, FW], bf16, tag="h1b")
                    nc.vector.tensor_mul(h1b[:], gt1[2][:], th1[:])
                    if t + 1 < L:
                        nc.vector.tensor_copy(hpart(tiles0[t + 1]), h1b[:])
                        # fan-out into the L2 input's x-part: cast-DMA off
                        # the compute engines (a full step of slack)
                        nc.gpsimd.dma_start(xpart(tiles1[t]), h1b[:])
                    else:
                        nc.vector.tensor_copy(xpart(tiles1[t]), h1b[:])

                # --- L2(t-1) acts + cell; tanh/h deferred ---
                if do2:
                    gt2 = gate_acts(1, ps2)
                    cell_pre(1, gt2)
                    h2dst = hpart(tiles1[t]) if t < L else None
                    pend2 = (gt2[2][:], h2dst)
                    assert h2dst is not None or t == L
                    if not DEFER_T2:
                        flush_pend2()
                        pend2 = None

                if do2:
                    tiles1.pop(s2, None)
                tiles0.pop(t - 1, None)

            if pend2 is not None:
                flush_pend2()        # final L2 step's tanh/h/max
            nc.sync.dma_start(out_d[:], mp[:])

    nc.compile()
    return nc


def _prep_core_inputs(xe_y, st, bias_arr, zpad, core):
    """xe_y: (B, 2, L, D) sqrt-normalized embeddings (axis1: 0=q, 1=a)."""
    sl = slice(4 * core, 4 * core + 4)
    # chains: s=0..3 -> q items, s=4..7 -> a items
    ch = np.concatenate([xe_y[sl, 0], xe_y[sl, 1]], axis=0)    # (8, L, D)
    # densities outer(y_s, y_s) as (L, D, CH*D): dens[t, p, s*D+j]
    ch2 = ch.transpose(1, 0, 2)                                # (L, 8, D)
    dens = np.einsum('lsp,lsj->lpsj', ch2, ch2)                # (L, D, 8, D)
    dens = np.ascontiguousarray(dens).reshape(L, D, CH * D).astype(F8)
    return {"xdens": dens, "st": st, "bias": bias_arr, "zpad": zpad}


def kernel(q, a, embed, conv_w, conv_b, lin_w, lin_b):
    from concourse import bass_utils

    q = np.asarray(q); a = np.asarray(a)
    embed = np.asarray(embed, np.float32)
    conv_w = np.asarray(conv_w, np.float32)
    conv_b = np.asarray(conv_b, np.float32)
    lin_w = np.asarray(lin_w, np.float32)
    lin_b = np.asarray(lin_b, np.float32)

    # host: embedding gather + density normalization factors
    idx = np.stack([q, a], axis=1).astype(np.int64)            # (B, 2, L)
    xe = embed[idx].astype(np.float64)                         # (B, 2, L, D)
    dot = np.sum(xe * xe, axis=-1, keepdims=True) + 1e-4
    xe_y = (xe / np.sqrt(dot)).astype(np.float32)

    # host: Toeplitz band stationaries  lhsT[(l,g,dh)] = B^T,
    # B[w, w'] = W[dh, w'-w+1]  (3 diagonals)
    st = np.zeros((NL * 4 * 4, D, D), np.float32)
    for l in range(NL):
        for g in range(4):
            W = conv_w[l, g, 0, 0]                             # (4, 3)
            for dh in range(4):
                Bm = sum(W[dh, dw] * np.eye(D, k=dw - 1) for dw in range(3))
                st[(l * 4 + g) * 4 + dh] = Bm.T.astype(np.float32)
    # dh-pair layout for DoubleRow: slot k=(l,g,pr) -> (D, two=2, D)
    stp = st.reshape(NL * 4 * 2, 2, D, D).transpose(0, 2, 1, 3)
    st = np.ascontiguousarray(stp).reshape(NL * 4 * 2, D, 2 * D).astype(F8)
    zpad = np.zeros((D, CH * 2), F8)
    bias_arr = np.tile(conv_b.reshape(1, -1), (D, 1)).astype(np.float32)

    in_maps = [_prep_core_inputs(xe_y, st, bias_arr, zpad, i)
               for i in range(NCORES)]
    _CACHE["in_maps"] = in_maps

    # every mp_out entry is max_t sigmoid*tanh, so |mp_out| < 1: a violation
    # means a bad run; rebuild (fresh schedule) and retry
    for attempt in range(3):
        if "nc" not in _CACHE:
            _CACHE["nc"] = _build_nc()
        nc = _CACHE["nc"]
        res = bass_utils.run_bass_kernel_spmd(
            nc, in_maps, core_ids=list(range(NCORES)))
        outs = [np.asarray(res.results[i]["mp_out"]).astype(np.float32)
                for i in range(NCORES)]
        if all(np.isfinite(o).all() and np.abs(o).max() <= 1.01 for o in outs):
            break
        _CACHE.pop("nc", None)

    # host: unshard + final linear + log_softmax
    q_p = np.zeros((B, D * D), np.float32)
    a_p = np.zeros((B, D * D), np.float32)
    for i in range(NCORES):
        out = outs[i]
        for s in range(CH):
            mp_T = out[:, s * D:(s + 1) * D]                   # (w, j)
            flat = np.ascontiguousarray(mp_T.T).reshape(-1)    # j-major
            if s < 4:
                q_p[4 * i + s] = flat
            else:
                a_p[4 * i + s - 4] = flat
    qa = np.concatenate([q_p, a_p], axis=1)
    score = qa @ lin_w.T + lin_b
    m = score.max(axis=1, keepdims=True)
    ls = score - m
    lse = np.log(np.exp(ls).sum(axis=1, keepdims=True))
    return (ls - lse).astype(np.float32)


# revision 73
# speedup vs baseline: 1.4728x; 1.4728x over previous
"""Trainium2 Bass kernel for NnqlmCnnBasedLstm.

Math (per batch item, per input sequence q/a):
  xe = embed[idx]                      (L, D)       D = 128
  dens_t = outer(xe_t, xe_t)/(|xe_t|^2 + 1e-4)     (D, D), symmetric
  2-layer ConvLSTM over L=40 steps; each gate g:
    pre_g = conv2d([xt; h], W_g, stride=(2,1), pad=(1,1)) + b_g  on (2D, D) -> (D, D)
  c = sig(f)*c + sig(i)*tanh(cc); h = sig(o)*tanh(c)
  out = max_t h2_t  -> flatten -> concat(q,a) -> linear(2) -> log_softmax

Device strategy (8 cores, data parallel over B=32 -> 4 items/core, each with a
q-chain and an a-chain = 8 chains/core):
  * State kept TRANSPOSED: tiles are (w partitions, j free); densities are
    symmetric so layer-1 inputs need no transpose.
  * conv: out_T[w, j] = sum_{dh,dw} W[dh,dw] * inp_T[w-1+dw, 2j-1+dh].
    For each dh this is a 3-diagonal Toeplitz band matrix (over w) applied on
    the TensorEngine; the four dh taps are packed as two fp8 DoubleRow
    matmuls (dh pairs (0,1) and (2,3) read 16-bit-aligned byte pairs of the
    input, the paired band matrices are the stationary).
  * Software pipeline: layer 2 lags layer 1 by one time step, so every
    cross-engine dependency has about a full step of slack and the PE
    streams matmuls gap-free (HAM stays at full clock).
  * sigmoid/tanh on ScalarE in full-width (1024 col) instructions; cell
    updates on VectorE in bf16; densities via a DMA row-broadcast of the
    embedding vectors plus per-chain VectorE tensor_scalar outer products;
    h1 fan-out copy on GpSimd.
  * Embedding gather, final linear + log_softmax on host (tiny).
"""

import os
import sys

import numpy as np
import ml_dtypes

for _p in ("/opt/trn_rl_repo", "/root/.axon_site/_ro/trn_rl_repo"):
    if os.path.isdir(_p) and _p not in sys.path:
        sys.path.insert(0, _p)

BF16 = np.dtype(ml_dtypes.bfloat16)
F8 = np.dtype(ml_dtypes.float8_e4m3)
DEFER_T2 = os.environ.get("KERNEL_DEFER_T2", "0") == "1"
SIM_MM = os.environ.get("KERNEL_SIM_MM", "0") == "1"

B, L, D, V, NL = 32, 40, 128, 32000, 2
NCORES = 8
CH = 8            # chains per core: 4 batch items x {q, a}
GW = 4            # chains per matmul group (psum free-width limit)
SEG = 258         # [z x(128) h(128) z]; dh-pair reads start at even elems
XOFF, HOFF = 1, 129
NF = CH * SEG
GF = GW * D       # free width of one matmul group (512)
FW = CH * D       # full free width (1024)

_CACHE = {}


def _build_nc(L=L):
    import concourse.bass as bass
    import concourse.bacc as bacc
    import concourse.mybir as mybir
    from concourse import tile

    f32 = mybir.dt.float32
    bf16 = mybir.dt.bfloat16
    f8 = mybir.dt.float8e4
    AF = mybir.ActivationFunctionType
    ALU = mybir.AluOpType
    DR = mybir.MatmulPerfMode.DoubleRow

    nc = bacc.Bacc(None, target_bir_lowering=False)

    # host-precomputed densities outer(y_s, y_s): (L, D, CH*D) fp8
    xdens_d = nc.dram_tensor("xdens", (L, D, FW), f8, kind="ExternalInput")
    # band stationaries as dh-pairs for DoubleRow: slot k=(l*4+g)*2+pr holds
    # [B_{2pr}^T ; B_{2pr+1}^T] as (D, two, D)
    st_d = nc.dram_tensor("st", (NL * 4 * 2, D, 2 * D), f8, kind="ExternalInput")
    bias_d = nc.dram_tensor("bias", (D, NL * 4), f32, kind="ExternalInput")
    zpad_d = nc.dram_tensor("zpad", (D, CH * 2), f8, kind="ExternalInput")
    out_d = nc.dram_tensor("mp_out", (D, FW), bf16, kind="ExternalOutput")

    GORDER = [3, 1, 0, 2]          # conv_w gate order: cc, i, f, o
    GTAG = {2: "po", 0: "pf", 1: "pi", 3: "pc"}

    with tile.TileContext(nc) as tc:
        with (
            tc.tile_pool(name="const", bufs=1) as constp,
            tc.tile_pool(name="state", bufs=1) as statep,
            tc.tile_pool(name="inp0", bufs=2) as inp0p,
            tc.tile_pool(name="inp1", bufs=2) as inp1p,
            tc.tile_pool(name="gate", bufs=2) as gatep,
            tc.tile_pool(name="psum", bufs=1, space="PSUM") as psump,
        ):
            # ---- constants ----
            stT = constp.tile([D, NL * 4 * 2 * 2 * D], f8, tag="stT")
            for i in range(NL * 4 * 2):
                nc.sync.dma_start(stT[:, i * 2 * D:(i + 1) * 2 * D], st_d[i])
            bias = constp.tile([D, NL * 4], f32, tag="bias")
            nc.sync.dma_start(bias[:], bias_d[:])

            # ---- persistent state ----
            c_l = [statep.tile([D, FW], bf16, tag=f"c{l}", name=f"c{l}")
                   for l in range(NL)]
            mp = statep.tile([D, FW], bf16, tag="mp")
            for l in range(NL):
                nc.vector.memset(c_l[l][:], 0.0)
            nc.vector.memset(mp[:], -1e30)

            def seg3(t):
                return t[:].rearrange("p (s c) -> p s c", s=CH)

            def xpart(t):
                return seg3(t)[:, :, XOFF:XOFF + D]

            def hpart(t):
                return seg3(t)[:, :, HOFF:HOFF + D]

            def new_inp(pool, tag, zero_pads):
                t = pool.tile([D, NF], f8, tag=tag, name=tag)
                if zero_pads:
                    v = seg3(t)
                    nc.sync.dma_start(v[:, :, 0:1], zpad_d[:, 0:CH])
                    nc.sync.dma_start(v[:, :, SEG - 1:SEG], zpad_d[:, 0:CH])
                return t

            def densities(t, dst):
                """dst x-part <- host-precomputed outer(y_s, y_s) via DMA."""
                nc.sync.dma_start(xpart(dst), xdens_d[t])

            def band_mms(l, inp):
                """Gate pre-activation DoubleRow matmuls, both chain groups."""
                # (p, two, s, c): elem = s*SEG + 2c + two; output j for pair
                # pr reads elems (2j+2pr, 2j+2pr+1)
                i4p = seg3(inp).rearrange("p s (c two) -> p two s c", two=2)
                ps = {}
                for g in GORDER:
                    p = psump.tile([D, FW], f32, tag=GTAG[g], name=GTAG[g])
                    ps[g] = p
                    for grp in range(2):
                        gs = slice(grp * GW, (grp + 1) * GW)
                        for pr in range(2):
                            k = (l * 4 + g) * 2 + pr
                            lhsT = stT[:, k * 2 * D:(k + 1) * 2 * D].rearrange(
                                "p (two m) -> p two m", two=2)
                            if not SIM_MM:
                                nc.tensor.matmul(
                                    p[:, grp * GF:(grp + 1) * GF],
                                    lhsT, i4p[:, :, gs, pr:pr + D],
                                    start=(pr == 0), stop=(pr == 1),
                                    perf_mode=DR,
                                )
                        if SIM_MM:
                            # per-chain 3D rhs (CoreSim-compatible); chain
                            # outer / pr inner so each 2KB psum zero region
                            # has one open accumulation group at a time
                            for s in range(GW):
                                ch = grp * GW + s
                                r3 = inp[:, ch * SEG:(ch + 1) * SEG] \
                                    .rearrange("p (c two) -> p two c", two=2)
                                for pr in range(2):
                                    k = (l * 4 + g) * 2 + pr
                                    lhsT = stT[:, k * 2 * D:(k + 1) * 2 * D] \
                                        .rearrange("p (two m) -> p two m",
                                                   two=2)
                                    nc.tensor.matmul(
                                        p[:, ch * D:(ch + 1) * D],
                                        lhsT, r3[:, :, pr:pr + D],
                                        start=(pr == 0), stop=(pr == 1),
                                        perf_mode=DR,
                                    )
                return ps

            def gate_acts(l, ps):
                gt = {}
                for g, af in ((3, AF.Tanh), (1, AF.Sigmoid),
                              (0, AF.Sigmoid), (2, AF.Sigmoid)):
                    dst = gatep.tile([D, FW], bf16, tag=f"g{g}", name=f"g{g}")
                    nc.scalar.activation(dst[:], ps[g][:], af,
                                         bias=bias[:, l * 4 + g:l * 4 + g + 1])
                    gt[g] = dst
                return gt

            def cell_pre(l, gt):
                t1 = gatep.tile([D, FW], bf16, tag="t1")
                t2 = gatep.tile([D, FW], bf16, tag="t2")
                nc.vector.tensor_mul(t2[:], gt[1][:], gt[3][:])
                nc.vector.tensor_mul(t1[:], gt[0][:], c_l[l][:])
                nc.vector.tensor_add(c_l[l][:], t1[:], t2[:])

            def cell_tanh(l):
                th = gatep.tile([D, FW], bf16, tag="th")
                nc.scalar.activation(th[:], c_l[l][:], AF.Tanh)
                return th

            # ---- prologue ----
            t0 = new_inp(inp0p, "inp0", True)         # L1 input for step 0
            nc.vector.memset(hpart(t0), 0.0)          # h1_{-1} = 0
            densities(0, t0)
            tiles0 = {0: t0}
            tiles1 = {}

            # ---- wall-clock loop: L1 computes step t, L2 computes t-1;
            # L2's tanh/h-store is deferred to the NEXT wall step ----
            pend2 = None             # (og tile, h2 dest ap) for L2 step t-2

            def flush_pend2():
                """tanh + h2 store + maxpool for the pending L2 step."""
                og2, h2dst = pend2
                th2 = cell_tanh(1)
                h2b = gatep.tile([D, FW], bf16, tag="h2b")
                nc.vector.tensor_mul(h2b[:], og2, th2[:])
                nc.vector.tensor_tensor(mp[:], mp[:], h2b[:], op=ALU.max)
                if h2dst is not None:
                    # fp8 cast into the L2 input tile (read next wall-step)
                    nc.vector.tensor_copy(h2dst, h2b[:])

            for t in range(L + 1):
                do1 = t < L          # L1 window for step t
                do2 = t >= 1         # L2 window for step t - 1
                s2 = t - 1

                if do1:
                    tiles1[t] = new_inp(inp1p, "inp1", True)
                    if t == 0:
                        nc.vector.memset(hpart(tiles1[0]), 0.0)  # h2_{-1}=0

                if pend2 is not None and DEFER_T2:
                    flush_pend2()
                    pend2 = None

                if do1 and t + 1 < L:
                    tiles0[t + 1] = new_inp(inp0p, "inp0", True)
                    densities(t + 1, tiles0[t + 1])

                # --- PE: L1(t) then L2(t-1) ---
                ps1 = band_mms(0, tiles0[t]) if do1 else None
                ps2 = band_mms(1, tiles1[s2]) if do2 else None

                # --- L1(t) acts + cell + h1 ---
                if do1:
                    gt1 = gate_acts(0, ps1)
                    cell_pre(0, gt1)
                    th1 = cell_tanh(0)
                    h1b = gatep.tile([D, FW], bf16, tag="h1b")
                    nc.vector.tensor_mul(h1b[:], gt1[2][:], th1[:])
                    if t + 1 < L:
                        nc.vector.tensor_copy(hpart(tiles0[t + 1]), h1b[:])
                        # fan-out into the L2 input's x-part: cast-DMA off
                        # the compute engines (a full step of slack)
                        nc.gpsimd.dma_start(xpart(tiles1[t]), h1b[:])
                    else:
                        nc.vector.tensor_copy(xpart(tiles1[t]), h1b[:])

                # --- L2(t-1) acts + cell; tanh/h deferred ---
                if do2:
                    gt2 = gate_acts(1, ps2)
                    cell_pre(1, gt2)
                    h2dst = hpart(tiles1[t]) if t < L else None
                    pend2 = (gt2[2][:], h2dst)
                    assert h2dst is not None or t == L
                    if not DEFER_T2:
                        flush_pend2()
                        pend2 = None

                if do2:
                    tiles1.pop(s2, None)
                tiles0.pop(t - 1, None)

            if pend2 is not None:
                flush_pend2()        # final L2 step's tanh/h/max
            nc.sync.dma_start(out_d[:], mp[:])

    nc.compile()
    return nc


def _prep_core_inputs(xe_y, st, bias_arr, zpad, core):
    """xe_y: (B, 2, L, D) sqrt-normalized embeddings (axis1: 0=q, 1=a)."""
    sl = slice(4 * core, 4 * core + 4)
    # chains: s=0..3 -> q items, s=4..7 -> a items
    ch = np.concatenate([xe_y[sl, 0], xe_y[sl, 1]], axis=0)    # (8, L, D)
    # densities outer(y_s, y_s) as (L, D, CH*D): dens[t, p, s*D+j]
    ch2 = ch.transpose(1, 0, 2)                                # (L, 8, D)
    dens = np.einsum('lsp,lsj->lpsj', ch2, ch2)                # (L, D, 8, D)
    dens = np.ascontiguousarray(dens).reshape(L, D, CH * D).astype(F8)
    return {"xdens": dens, "st": st, "bias": bias_arr, "zpad": zpad}


def kernel(q, a, embed, conv_w, conv_b, lin_w, lin_b):
    from concourse import bass_utils

    q = np.asarray(q); a = np.asarray(a)
    embed = np.asarray(embed, np.float32)
    conv_w = np.asarray(conv_w, np.float32)
    conv_b = np.asarray(conv_b, np.float32)
    lin_w = np.asarray(lin_w, np.float32)
    lin_b = np.asarray(lin_b, np.float32)

    # host: embedding gather + density normalization factors
    idx = np.stack([q, a], axis=1).astype(np.int64)            # (B, 2, L)
    xe = embed[idx].astype(np.float64)                         # (B, 2, L, D)
    dot = np.sum(xe * xe, axis=-1, keepdims=True) + 1e-4
    xe_y = (xe / np.sqrt(dot)).astype(np.float32)

    # host: Toeplitz band stationaries  lhsT[(l,g,dh)] = B^T,
    # B[w, w'] = W[dh, w'-w+1]  (3 diagonals)
    st = np.zeros((NL * 4 * 4, D, D), np.float32)
    for l in range(NL):
        for g in range(4):
            W = conv_w[l, g, 0, 0]                             # (4, 3)
            for dh in range(4):
                Bm = sum(W[dh, dw] * np.eye(D, k=dw - 1) for dw in range(3))
                st[(l * 4 + g) * 4 + dh] = Bm.T.astype(np.float32)
    # dh-pair layout for DoubleRow: slot k=(l,g,pr) -> (D, two=2, D)
    stp = st.reshape(NL * 4 * 2, 2, D, D).transpose(0, 2, 1, 3)
    st = np.ascontiguousarray(stp).reshape(NL * 4 * 2, D, 2 * D).astype(F8)
    zpad = np.zeros((D, CH * 2), F8)
    bias_arr = np.tile(conv_b.reshape(1, -1), (D, 1)).astype(np.float32)

    in_maps = [_prep_core_inputs(xe_y, st, bias_arr, zpad, i)
               for i in range(NCORES)]
    _CACHE["in_maps"] = in_maps

    # every mp_out entry is max_t sigmoid*tanh, so |mp_out| < 1: a violation
    # means a bad run; rebuild (fresh schedule) and retry
    for attempt in range(3):
        if "nc" not in _CACHE:
            _CACHE["nc"] = _build_nc()
        nc = _CACHE["nc"]
        res = bass_utils.run_bass_kernel_spmd(
            nc, in_maps, core_ids=list(range(NCORES)))
        outs = [np.asarray(res.results[i]["mp_out"]).astype(np.float32)
                for i in range(NCORES)]
        if all(np.isfinite(o).all() and np.abs(o).max() <= 1.01 for o in outs):
            break
        _CACHE.pop("nc", None)

    # host: unshard + final linear + log_softmax
    q_p = np.zeros((B, D * D), np.float32)
    a_p = np.zeros((B, D * D), np.float32)
    for i in range(NCORES):
        out = outs[i]
        for s in range(CH):
            mp_T = out[:, s * D:(s + 1) * D]                   # (w, j)
            flat = np.ascontiguousarray(mp_T.T).reshape(-1)    # j-major
            if s < 4:
                q_p[4 * i + s] = flat
            else:
                a_p[4 * i + s - 4] = flat
    qa = np.concatenate([q_p, a_p], axis=1)
    score = qa @ lin_w.T + lin_b
    m = score.max(axis=1, keepdims=True)
    ls = score - m
    lse = np.log(np.exp(ls).sum(axis=1, keepdims=True))
    return (ls - lse).astype(np.float32)
